# revision 11
# baseline (speedup 1.0000x reference)
"""Trainium2 Bass kernel for nn_Decoder (attention decoder step + LSTM + generator).

Distribution over 8 NeuronCores:
  - Phase A (attention over encoder_out): data-parallel over batch B=64
    (8 rows per core). The reference's proj/scores einsum pair is folded:
        scores[s,b] = enc[s,b,:] . q[b,:],   q = h0 @ W_in
    so encoder_out (512 MB) is streamed exactly once per shard, f32.
    Scores: fused DVE multiply+reduce (scalar_tensor_tensor + accum_out), f32.
    Softmax: gpsimd partition_all_reduce + DVE reduce + ScalarE exp w/ accum.
    Context: TensorE bf16 matmuls (ScalarE downcasts encoder tiles) with the
    unnormalized softmax weights stationary; normalized at PSUM->SBUF copy.
    The per-row softmax/context work is software-pipelined one iteration
    behind the score pass so DVE/PE/ACT overlap across batch rows.
  - Phase B: tensor-parallel, bf16 weights. context is all-gathered
    ([64,1024], 32KB/core); W_out applied replicated; LSTM gate weights
    W_ih/W_hh sharded by output feature (each core computes a 128-wide
    h-slice of all 4 gates); hN slices all-gathered; generator W_gen
    vocab-sharded (2944 padded rows/core). Embedding lookup is an
    indirect-DMA gather done on every core.

kernel(**inputs) takes the full unsharded inputs and returns
(logits [64,23262], hN [64,1024], cN [64,1024]) like the reference.
"""
import sys
import os
import numpy as np

KSTOP = int(os.environ.get("KSTOP", "5"))
NB = int(os.environ.get("NB", "8"))
PBSTOP = int(os.environ.get("PBSTOP", "5"))

sys.path.insert(0, "/opt/trn_rl_repo")

import concourse.bass as bass
import concourse.bacc as bacc
import concourse.tile as tile
import concourse.mybir as mybir
import concourse.bass_isa as bass_isa
import concourse.bass_utils as bass_utils

F32 = mybir.dt.float32
BF16 = mybir.dt.bfloat16
AL = mybir.AluOpType
AF = mybir.ActivationFunctionType

N_CORES = 8
V, E, H = 23262, 300, 1024
S, B = 2048, 64
BL = B // N_CORES          # local batch rows per core (8)
HS = H // N_CORES          # hidden slice per core (128)
VP = 23552                 # vocab padded to 184*128
VC = VP // N_CORES         # vocab rows per core (2944)
NVC = VC // 128            # vocab chunks per core (23)
EP = 384                   # embedding dim padded
KIH = H + EP               # padded W_ih contraction dim (1408)
NCI = S // 128             # encoder s-chunks per batch row (16)
NKH = H // 128             # 8
NWGA = 2                   # W_gen k-tiles prefetched during phase A


def _build():
    nc = bacc.Bacc("TRN2", num_devices=N_CORES, debug=False, enable_asserts=False)

    # ---- per-core DRAM inputs
    enc_d = nc.dram_tensor("enc", [BL, S, H], F32, kind="ExternalInput")
    h0T_d = nc.dram_tensor("h0T", [H, B], BF16, kind="ExternalInput")
    h0Tl_d = nc.dram_tensor("h0Tl", [H, BL], F32, kind="ExternalInput")
    c0T_d = nc.dram_tensor("c0T", [HS, B], F32, kind="ExternalInput")
    win_d = nc.dram_tensor("win", [H, H], F32, kind="ExternalInput")
    woutT_d = nc.dram_tensor("woutT", [2 * H, H], BF16, kind="ExternalInput")
    wihT_d = nc.dram_tensor("wihT", [KIH, 4 * HS], BF16, kind="ExternalInput")
    whhT_d = nc.dram_tensor("whhT", [H, 4 * HS], BF16, kind="ExternalInput")
    bias4_d = nc.dram_tensor("bias4", [HS, 4], F32, kind="ExternalInput")
    wgenT_d = nc.dram_tensor("wgenT", [H, VC], BF16, kind="ExternalInput")
    bgen_d = nc.dram_tensor("bgen", [128, NVC], F32, kind="ExternalInput")
    emb_d = nc.dram_tensor("emb", [VP, EP], F32, kind="ExternalInput")
    tok_d = nc.dram_tensor("tok", [B, 1], mybir.dt.int32, kind="ExternalInput")
    ident_d = nc.dram_tensor("ident", [128, 128], F32, kind="ExternalInput")

    # ---- per-core DRAM outputs
    logitsT_d = nc.dram_tensor("logitsT", [NVC, 128, B], F32, kind="ExternalOutput")
    hNT_d = nc.dram_tensor("hNT", [HS, B], F32, kind="ExternalOutput")
    cNT_d = nc.dram_tensor("cNT", [HS, B], F32, kind="ExternalOutput")

    groups = [list(range(N_CORES))]

    with tile.TileContext(nc) as tc:
        with (
            tc.tile_pool(name="const", bufs=1) as cp,
            tc.tile_pool(name="work", bufs=2) as wp,
            tc.tile_pool(name="wgenA", bufs=1) as wgpA,
            tc.tile_pool(name="dram", bufs=1, space="DRAM") as dp,
        ):
            # ================= constants =================
            ident = cp.tile([128, 128], F32, tag="ident")
            nc.sync.dma_start(ident[:], ident_d[:])
            h0T = cp.tile([128, NKH, B], BF16, tag="h0T")
            nc.sync.dma_start(h0T[:], h0T_d.rearrange("(kc p) b -> p kc b", p=128))
            h0Tl = cp.tile([128, NKH, BL], F32, tag="h0Tl")
            nc.sync.dma_start(h0Tl[:], h0Tl_d.rearrange("(kc p) b -> p kc b", p=128))
            c0T = cp.tile([HS, B], F32, tag="c0T")
            nc.sync.dma_start(c0T[:], c0T_d[:])
            bias4 = cp.tile([HS, 4], F32, tag="bias4")
            nc.sync.dma_start(bias4[:], bias4_d[:])
            bgen = cp.tile([128, NVC], F32, tag="bgen")
            nc.sync.dma_start(bgen[:], bgen_d[:])

            # W_gen k-tiles 0..NWGA-1 prefetched during phase A
            wg_t = []
            for kc in range(NWGA):
                wg = wgpA.tile([128, VC], BF16, tag=f"wg{kc}", name=f"wg{kc}")
                nc.sync.dma_start(wg[:], wgenT_d[kc * 128:(kc + 1) * 128, :])
                wg_t.append(wg)

            # DRAM bounce buffers
            qdram = dp.tile([BL, H], F32, tag="qdram")
            ag_in = dp.tile([BL, H], F32, tag="ag_in")
            ag_out = dp.tile([B, H], F32, tag="ag_out")
            hn_b = dp.tile([HS, B], F32, tag="hn_b")
            hn_ag = dp.tile([N_CORES, HS, B], F32, tag="hn_ag")

            # ================= embedding gather =================
            tok = cp.tile([B, 1], mybir.dt.int32, tag="tok")
            nc.sync.dma_start(tok[:], tok_d[:])
            emb_sb = cp.tile([B, EP], F32, tag="emb_sb")
            nc.gpsimd.indirect_dma_start(
                out=emb_sb[:], out_offset=None, in_=emb_d[:],
                in_offset=bass.IndirectOffsetOnAxis(ap=tok[:, :1], axis=0),
            )
            embT = cp.tile([128, EP // 128, B], BF16, tag="embT")

            with tc.tile_pool(name="psA", bufs=1, space="PSUM") as psA, \
                 tc.tile_pool(name="win", bufs=2) as winp:
                # embT transposes (through PSUM)
                for j in range(EP // 128):
                    tp = psA.tile([128, B], F32, tag="small")
                    nc.tensor.transpose(
                        out=tp[:], in_=emb_sb[:, j * 128:(j + 1) * 128],
                        identity=ident[:B, :B])
                    nc.vector.tensor_copy(embT[:, j, :], tp[:])

                # ============ q = h0_local @ W_in, broadcast ============
                q_ps = psA.tile([BL, H], F32, tag="qps")
                for kc in range(NKH):
                    wi = winp.tile([128, H], F32, tag="wi")
                    nc.sync.dma_start(wi[:], win_d[kc * 128:(kc + 1) * 128, :])
                    for nh in range(2):
                        nc.tensor.matmul(
                            q_ps[:, nh * 512:(nh + 1) * 512],
                            h0Tl[:, kc, :], wi[:, nh * 512:(nh + 1) * 512],
                            start=(kc == 0), stop=(kc == NKH - 1))
                q_sb = wp.tile([BL, H], F32, tag="q_sb", bufs=1)
                nc.vector.tensor_copy(q_sb[:], q_ps[:])
                # bounce through DRAM to get each q row onto partition 0
                nc.sync.dma_start(qdram[:], q_sb[:])

                # ================= phase A: attention ====================
                # software-pipelined: row b's softmax/context is emitted
                # after row b+1's score pass so engines overlap across rows.
                def softmax_ctx(b, scores, enc_bf):
                    prmax = wp.tile([128, NCI], F32, tag="prmax", name="prmax")
                    nc.gpsimd.partition_all_reduce(
                        prmax[:], scores[:], channels=128,
                        reduce_op=bass_isa.ReduceOp.max)
                    nmax = wp.tile([128, 1], F32, tag="nmax", name="nmax")
                    nc.vector.tensor_reduce(
                        out=nmax[:], in_=prmax[:], axis=mybir.AxisListType.X,
                        op=AL.max, negate=True)
                    e_sb = wp.tile([128, NCI], BF16, tag="e_sb", name="e_sb")
                    sume = wp.tile([128, 1], F32, tag="sume", name="sume")
                    nc.scalar.activation(
                        out=e_sb[:], in_=scores[:], func=AF.Exp,
                        bias=nmax[:], scale=1.0, accum_out=sume[:])
                    sden = wp.tile([128, 1], F32, tag="sden", name="sden")
                    nc.gpsimd.partition_all_reduce(
                        sden[:], sume[:], channels=128,
                        reduce_op=bass_isa.ReduceOp.add)
                    rden = wp.tile([128, 1], F32, tag="rden", name="rden")
                    nc.vector.reciprocal(rden[:], sden[:])
                    # context row: ctx[h] = (sum_s e[s] enc[s, h]) / den
                    ctx_ps = psA.tile([1, H], F32, tag="ctx", name="ctx_ps")
                    for ci in range(NCI):
                        for nh in range(2):
                            nc.tensor.matmul(
                                ctx_ps[:, nh * 512:(nh + 1) * 512],
                                e_sb[:, ci:ci + 1],
                                enc_bf[:, ci, nh * 512:(nh + 1) * 512],
                                start=(ci == 0), stop=(ci == NCI - 1))
                    ctx_row = wp.tile([1, H], F32, tag="ctx_row", name="ctx_row")
                    nc.scalar.activation(
                        out=ctx_row[:], in_=ctx_ps[:], func=AF.Copy,
                        scale=rden[0:1, :])
                    nc.sync.dma_start(ag_in[b:b + 1, :], ctx_row[:])

                with tc.tile_pool(name="enc", bufs=2) as encp:
                    pending = None
                    for b in range(min(BL, NB) if KSTOP >= 2 else 0):
                        # broadcast this row's query to all 128 partitions
                        q_row = wp.tile([1, H], F32, tag="q_row", bufs=1)
                        nc.sync.dma_start(q_row[:], qdram[b:b + 1, :])
                        q_bc = wp.tile([128, H], F32, tag="q_bc")
                        nc.gpsimd.partition_broadcast(q_bc[:], q_row[:])

                        enc_bf = encp.tile([128, NCI, H], BF16, tag="enc_bf",
                                           name="enc_bf")
                        scores = wp.tile([128, NCI], F32, tag="scores",
                                         name="scores")
                        src = enc_d[b].rearrange("(ci p) h -> p ci h", p=128)
                        for half in range(2):
                            enc_sb = encp.tile([128, NCI // 2, H], F32,
                                               tag="enc", name="enc_sb")
                            nc.sync.dma_start(
                                enc_sb[:], src[:, half * 8:(half + 1) * 8, :])
                            for cj in range(NCI // 2):
                                ci = half * 8 + cj
                                scr = wp.tile([128, H], F32, tag="scr",
                                              name="scr")
                                nc.vector.scalar_tensor_tensor(
                                    out=scr[:], in0=enc_sb[:, cj, :],
                                    scalar=1.0, in1=q_bc[:],
                                    op0=AL.bypass, op1=AL.mult,
                                    accum_out=scores[:, ci:ci + 1])
                                nc.scalar.activation(
                                    out=enc_bf[:, ci, :], in_=enc_sb[:, cj, :],
                                    func=AF.Copy)
                        if pending is not None:
                            softmax_ctx(*pending)
                        pending = (b, scores, enc_bf)
                    if pending is not None:
                        softmax_ctx(*pending)

            if PBSTOP >= 1:
                # ================= phase B =================
                nc.gpsimd.collective_compute(
                    "AllGather", AL.bypass, replica_groups=groups,
                    ins=[ag_in.opt()], outs=[ag_out.opt()])
                ctx_rows = cp.tile([B, H], F32, tag="ctx_rows")
                nc.sync.dma_start(ctx_rows[:], ag_out[:])

                ctxT = cp.tile([128, NKH, B], BF16, tag="ctxT")
                with tc.tile_pool(name="psB1", bufs=2, space="PSUM") as psB1:
                    for hb in range(NKH):
                        tp = psB1.tile([128, B], F32, tag="small")
                        nc.tensor.transpose(
                            out=tp[:], in_=ctx_rows[:, hb * 128:(hb + 1) * 128],
                            identity=ident[:B, :B])
                        nc.vector.tensor_copy(ctxT[:, hb, :], tp[:])

            # ctx_hat = tanh(W_out @ [ctx; h0]) -- full H on every core
            chT = cp.tile([128, NKH, B], BF16, tag="chT")
            if PBSTOP >= 2:
                with tc.tile_pool(name="psB2", bufs=1, space="PSUM") as psB2, \
                     tc.tile_pool(name="wout", bufs=3) as woutp:
                    ch_ps = [psB2.tile([128, B], F32, tag=f"ch{hb}", name=f"ch{hb}")
                             for hb in range(NKH)]
                    for kc in range(2 * NKH):
                        wo = woutp.tile([128, H], BF16, tag="wo")
                        nc.sync.dma_start(wo[:], woutT_d[kc * 128:(kc + 1) * 128, :])
                        rhs = ctxT[:, kc, :] if kc < NKH else h0T[:, kc - NKH, :]
                        for hb in range(NKH):
                            nc.tensor.matmul(
                                ch_ps[hb][:], wo[:, hb * 128:(hb + 1) * 128], rhs,
                                start=(kc == 0), stop=(kc == 2 * NKH - 1))
                    for hb in range(NKH):
                        nc.scalar.activation(out=chT[:, hb, :], in_=ch_ps[hb][:],
                                             func=AF.Tanh)

            # LSTM gates (this core's 128-wide slice of each gate)
            if PBSTOP >= 3:
                with tc.tile_pool(name="psB3", bufs=1, space="PSUM") as psB3, \
                     tc.tile_pool(name="wih", bufs=3) as wihp:
                    g_ps = [psB3.tile([HS, B], F32, tag=f"g{g}", name=f"g{g}")
                            for g in range(4)]
                    nkc_ih = KIH // 128  # 11
                    for kc in range(nkc_ih):
                        wih = wihp.tile([128, 4 * HS], BF16, tag="wih")
                        nc.sync.dma_start(wih[:], wihT_d[kc * 128:(kc + 1) * 128, :])
                        rhs = chT[:, kc, :] if kc < NKH else embT[:, kc - NKH, :]
                        for g in range(4):
                            nc.tensor.matmul(
                                g_ps[g][:], wih[:, g * HS:(g + 1) * HS], rhs,
                                start=(kc == 0), stop=False)
                    for kc in range(NKH):
                        whh = wihp.tile([128, 4 * HS], BF16, tag="wih")
                        nc.sync.dma_start(whh[:], whhT_d[kc * 128:(kc + 1) * 128, :])
                        for g in range(4):
                            nc.tensor.matmul(
                                g_ps[g][:], whh[:, g * HS:(g + 1) * HS],
                                h0T[:, kc, :],
                                start=False, stop=(kc == NKH - 1))

                    i_s = wp.tile([HS, B], F32, tag="i_s")
                    f_s = wp.tile([HS, B], F32, tag="f_s")
                    g_t = wp.tile([HS, B], F32, tag="g_t")
                    o_s = wp.tile([HS, B], F32, tag="o_s")
                    nc.scalar.activation(out=i_s[:], in_=g_ps[0][:],
                                         func=AF.Sigmoid, bias=bias4[:, 0:1])
                    nc.scalar.activation(out=f_s[:], in_=g_ps[1][:],
                                         func=AF.Sigmoid, bias=bias4[:, 1:2])
                    nc.scalar.activation(out=g_t[:], in_=g_ps[2][:],
                                         func=AF.Tanh, bias=bias4[:, 2:3])
                    nc.scalar.activation(out=o_s[:], in_=g_ps[3][:],
                                         func=AF.Sigmoid, bias=bias4[:, 3:4])

                tA = wp.tile([HS, B], F32, tag="tA")
                nc.vector.tensor_tensor(out=tA[:], in0=f_s[:], in1=c0T[:], op=AL.mult)
                tB = wp.tile([HS, B], F32, tag="tB")
                nc.vector.tensor_tensor(out=tB[:], in0=i_s[:], in1=g_t[:], op=AL.mult)
                cN = wp.tile([HS, B], F32, tag="cN")
                nc.vector.tensor_tensor(out=cN[:], in0=tA[:], in1=tB[:], op=AL.add)
                tcn = wp.tile([HS, B], F32, tag="tcn")
                nc.scalar.activation(out=tcn[:], in_=cN[:], func=AF.Tanh)
                hN = wp.tile([HS, B], F32, tag="hN")
                nc.vector.tensor_tensor(out=hN[:], in0=o_s[:], in1=tcn[:], op=AL.mult)
                nc.sync.dma_start(cNT_d[:], cN[:])
                nc.sync.dma_start(hNT_d[:], hN[:])
                nc.sync.dma_start(hn_b[:], hN[:])

            if PBSTOP >= 4:
                nc.gpsimd.collective_compute(
                    "AllGather", AL.bypass, replica_groups=groups,
                    ins=[hn_b.opt()], outs=[hn_ag.opt()])
                hNT_f = cp.tile([128, NKH, B], F32, tag="hNT_f")
                nc.sync.dma_start(hNT_f[:], hn_ag[:].rearrange("kc p b -> p kc b"))
                hNT = cp.tile([128, NKH, B], BF16, tag="hNT")
                nc.vector.tensor_copy(hNT[:], hNT_f[:])

            # generator: logits slice = W_gen_shard @ hN + b_gen_shard
            if PBSTOP >= 5:
                lgout = cp.tile([128, NVC, B], F32, tag="lgout")
                with tc.tile_pool(name="psB4", bufs=4, space="PSUM") as psB4, \
                     tc.tile_pool(name="wgenB", bufs=1) as wgpB:
                    for kc in range(NWGA, NKH):
                        wg = wgpB.tile([128, VC], BF16, tag=f"wg{kc}", name=f"wg{kc}")
                        nc.sync.dma_start(wg[:], wgenT_d[kc * 128:(kc + 1) * 128, :])
                        wg_t.append(wg)
                    for vc in range(NVC):
                        l_ps = psB4.tile([128, B], F32, tag="lps")
                        for kc in range(NKH):
                            nc.tensor.matmul(
                                l_ps[:], wg_t[kc][:, vc * 128:(vc + 1) * 128],
                                hNT[:, kc, :],
                                start=(kc == 0), stop=(kc == NKH - 1))
                        nc.scalar.activation(
                            out=lgout[:, vc, :], in_=l_ps[:], func=AF.Identity,
                            bias=bgen[:, vc:vc + 1])
                nc.sync.dma_start(
                    logitsT_d.rearrange("vc p b -> p vc b"), lgout[:])

    nc.compile()
    return nc


_NC = None


def _get_nc():
    global _NC
    if _NC is None:
        _NC = _build()
    return _NC


def _prep_inputs(tokens, encoder_out, h0, c0, emb, W_in, W_out, W_ih, W_hh,
                 b_ih, b_hh, W_gen, b_gen):
    f32 = np.float32
    tokens = np.asarray(tokens)
    encoder_out = np.asarray(encoder_out, f32)
    h0 = np.asarray(h0, f32)
    c0 = np.asarray(c0, f32)
    emb = np.asarray(emb, f32)
    W_in = np.ascontiguousarray(np.asarray(W_in, f32))
    W_out = np.asarray(W_out, f32)
    W_ih = np.asarray(W_ih, f32)
    W_hh = np.asarray(W_hh, f32)
    bb = np.asarray(b_ih, f32) + np.asarray(b_hh, f32)
    W_gen = np.asarray(W_gen, f32)
    b_gen = np.asarray(b_gen, f32)

    import ml_dtypes
    bf16 = ml_dtypes.bfloat16
    h0T = np.ascontiguousarray(h0.T).astype(bf16)
    woutT = np.ascontiguousarray(W_out.T).astype(bf16)
    embp = np.zeros((VP, EP), f32)
    embp[:V, :E] = emb
    tok = np.ascontiguousarray(tokens.astype(np.int32).reshape(B, 1))
    ident = np.eye(128, dtype=f32)

    wgenp = np.zeros((VP, H), f32)
    wgenp[:V] = W_gen
    bgenp = np.zeros((VP,), f32)
    bgenp[:V] = b_gen

    in_maps = []
    for c in range(N_CORES):
        enc_c = np.ascontiguousarray(
            encoder_out[:, c * BL:(c + 1) * BL, :].transpose(1, 0, 2))
        h0Tl = np.ascontiguousarray(h0[c * BL:(c + 1) * BL, :].T)
        c0T = np.ascontiguousarray(c0[:, c * HS:(c + 1) * HS].T)
        rows = np.concatenate(
            [W_ih[g * H + c * HS: g * H + (c + 1) * HS] for g in range(4)], 0)
        wihT = np.zeros((KIH, 4 * HS), f32)
        wihT[:H + E] = rows.T
        wihT = wihT.astype(bf16)
        rows_hh = np.concatenate(
            [W_hh[g * H + c * HS: g * H + (c + 1) * HS] for g in range(4)], 0)
        whhT = np.ascontiguousarray(rows_hh.T).astype(bf16)
        bias4 = np.stack(
            [bb[g * H + c * HS: g * H + (c + 1) * HS] for g in range(4)], 1)
        wgenT = np.ascontiguousarray(wgenp[c * VC:(c + 1) * VC].T).astype(bf16)
        bgen = np.ascontiguousarray(
            bgenp[c * VC:(c + 1) * VC].reshape(NVC, 128).T)
        in_maps.append({
            "enc": enc_c, "h0T": h0T, "h0Tl": h0Tl, "c0T": c0T,
            "win": W_in, "woutT": woutT, "wihT": wihT, "whhT": whhT,
            "bias4": np.ascontiguousarray(bias4), "wgenT": wgenT,
            "bgen": bgen, "emb": embp, "tok": tok, "ident": ident,
        })
    return in_maps


def kernel(**inputs):
    in_maps = _prep_inputs(**inputs)
    nc = _get_nc()
    res = bass_utils.run_bass_kernel_spmd(
        nc, in_maps, core_ids=list(range(N_CORES)))
    logits_parts, hN_parts, cN_parts = [], [], []
    for c in range(N_CORES):
        r = res.results[c]
        logits_parts.append(r["logitsT"].reshape(VC, B).T)
        hN_parts.append(r["hNT"].T)
        cN_parts.append(r["cNT"].T)
    logits = np.concatenate(logits_parts, axis=1)[:, :V]
    hN = np.concatenate(hN_parts, axis=1)
    cN = np.concatenate(cN_parts, axis=1)
    return logits, hN, cN
